# revision 22
# baseline (speedup 1.0000x reference)
"""Trainium2 Bass kernel for nn_Attention (channel-attention, 8 NeuronCores).

Algorithm (algebraically identical to the reference):
  The attention contracts over the spatial axis n = 32*32*32 = 32768, and the
  attention matrices are tiny (64x64 per head).  Everything collapses around
  the per-batch Gram matrix G_b = x_b @ x_b^T (128x128):

    scores_bh = scale * Wq_h G_b Wk_h^T            (tiny)
    attn      = softmax(scores)                     (tiny)
    W_eff_b   = (1/n) * sum_h Wout_h attn_bh Wv_h   (64x128, tiny)
    y_b       = W_eff_b @ x_b + b_out               (the only other big matmul)

  Sharding: NO collectives (an ncfw collective costs 60-80us of firmware
  wakeup on this stack, dwarfing the kernel).  Every core receives the FULL
  x in fp8-e4m3 [n, c] layout (8 MB) and computes the complete Gram
  redundantly (fp8 is harmless: the Gram contracts over 32768 samples), plus
  its own 1/8 spatial shard in bf16 [c, n] layout (2 MB) for the y matmul.

  Performance model (from NTFF/perfetto analysis, iterated on HW):
  - The input stream is the wall: ~10.9 MB at ~360 GB/s = ~30 us/core.
  - The PE consumes a fp8 SwInterleave Gram pair (256 spatial rows) every
    ~80 ns (the production LDWEIGHTS+MATMUL issue floor at N=128), i.e.
    ~20.7 us of Gram work that must ride inside the stream.
  - The HW power manager grants the PE only ~20 us of full clock per run
    (HAM k=8/8 windows in the NTFF), then duty-throttles to half; every
    wasted PE cycle stretches the run at 2x.  So: no filler matmuls, and
    total PE work is kept minimal.
  Structure:
  - 256KB-piece streaming of ALL gram bytes first (b0 then b1 back to
    back), then the xc shards, so the tail-needed data arrives last and
    the PE is never starved at a batch boundary.  All input descriptors
    ride the sync ring ALONE: one HWDGE ring sustains the full stream,
    and the scalar queue stays descriptor-free -- otherwise its ACT exps
    sit behind ring-backpressured issues until the whole stream has been
    enqueued, stalling the batch-0 attention chain (measured +5 us).
  - Batch-1's Gram is split [chunks 0-6 | chunk 7] into two PSUM tiles so
    a1 = G Wq for the big part runs during the last chunk's data wait.
  - The softmax is pipelined per head-group: ACT exp carries bias=-max
    and accum_out=row-sum in ONE instruction, so the chain per group is
    DVE(max) -> ACT(exp+sum) -> DVE(recip, scale) -> PE(mt).
  - W_eff is computed monolithically (8 quadrant matmuls, one [128,256]
    cast, 4 accumulating matmuls): two cross-engine hops total.
  - Phase E packs two 512-col output chunks into one [128, 512] PSUM tile
    (PE quadrant packing), halving the bias-add and output-DMA count; the
    bias-adds alternate ACT/DVE so consecutive pairs parallelize, and
    batch-0's adds are dependency-gated behind the batch-1 exp chain
    (bo_late bypass) so the scheduler cannot wedge them into it.
  - Both phase-E's run on the tail, where weff0+E0 cover the softmax1
    ACT/DVE chain and the xc shards arrive exactly as consumed.
"""

import numpy as np
import ml_dtypes

import concourse.bass as bass
import concourse.bacc as bacc
import concourse.mybir as mybir
import concourse.tile as tile
from concourse.bass_utils import run_bass_kernel_spmd

NCORES = 8
P = 128
N_TOT = 32 * 32 * 32          # 32768 spatial points
NSH = N_TOT // NCORES         # 4096 per core per batch (output shard)
SUB = N_TOT // P              # 256 fp8 k-subtiles per batch
CHUNK_SUB = 32                # subtiles per DMA chunk (512 KB)
NCHUNK = SUB // CHUNK_SUB     # 8 chunks per batch
CHW = CHUNK_SUB * P           # 4096 fp8 free columns per chunk
PIECES = 2                    # DMA pieces per chunk (256 KB each)
HEADS = 8
DH = 64
SCALE = DH ** -0.5
WCOLS = 512 + 512 + 512 + 256 + 1  # packed weights: wq|wk|wv|wo|bo
WARM_START = 0                # PE warm-keepers (OFF: the HW throttle is a
WARM_MID = 0                  # utilization budget -- idle EARNS credit, so
WARM_TAIL = 0                 # fillers burn it and stretch the run)
BF = mybir.dt.bfloat16
F32 = mybir.dt.float32
FP8 = mybir.dt.float8e4
DR = mybir.MatmulPerfMode.DoubleRow
DRSW = mybir.MatmulPerfMode.DoubleRowSwInterleave
EXP = mybir.ActivationFunctionType.Exp
bf16 = ml_dtypes.bfloat16
f8 = ml_dtypes.float8_e4m3

_CACHED_NC = None


class _TrimmedTileContext(tile.TileContext):
    """TileContext with a minimal exit sequence.

    The stock exit is drain -> barrier -> sem-clear -> barrier; the
    barrier + clear lower to an EVSEM butterfly measured at ~7us (every
    engine walks the 27-sem global clock).  For a single-shot kernel the
    Sync drain with global-clock waits already gates completion on every
    DMA and engine op, each engine halts in-order after its last
    scheduled instruction, and the engine preamble re-initializes the
    semaphore file on the next execution (verified: back-to-back
    executions of the same loaded NEFF stay correct).  So keep only the
    drain.
    """

    def _drain_and_barrier(self, tick_clock, wait_clock):
        from concourse.vector_clock import ScopedClock

        drain_inst = self.nc.sync.drain()
        wait_clock.add_sem_waits(
            drain_inst.ins, ScopedClock({None: tick_clock.global_clock})
        )
        popped = self.nc._tile_sem_poison_stack.pop()
        assert popped is self._sem_poison


def build_nc():
    # The stock Bass init ends with const-AP memsets guarded by a second
    # all-engine barrier; the consts are unused here and the barrier adds
    # ~2us of start-up serialization, so skip that one barrier only.
    orig_barrier = bass.Bass.all_engine_barrier
    bass.Bass.all_engine_barrier = lambda self: None
    try:
        nc = bacc.Bacc(
            "TRN2", target_bir_lowering=False, debug=False, num_devices=NCORES
        )
    finally:
        bass.Bass.all_engine_barrier = orig_barrier

    # full x, fp8, [p, (b, m, c)] DoubleRow layout: subtile m holds spatial
    # rows m*128..m*128+127 of batch b, channels on the innermost axis.
    xg_ext = nc.dram_tensor("xg", [P, 2 * SUB * P], FP8, kind="ExternalInput")
    # own output shard, bf16, [c, (b, n)] layout for the y matmul
    xc_ext = nc.dram_tensor("xc", [P, 2 * NSH], BF, kind="ExternalInput")
    w_ext = nc.dram_tensor("wpack", [P, WCOLS], BF, kind="ExternalInput")
    # y out, bf16: partition = (chunk-half, row), free = (b, pair, 512)
    out_ext = nc.dram_tensor("out", [P, NSH], BF, kind="ExternalOutput")

    with _TrimmedTileContext(nc) as tc:
        with (
            tc.tile_pool(name="const", bufs=1) as const,
            tc.tile_pool(name="data", bufs=1) as data,
            tc.tile_pool(name="work", bufs=1) as work,
            tc.tile_pool(name="ypool", bufs=8) as ypool,
            tc.tile_pool(name="psg", bufs=2, space="PSUM") as psg,
            tc.tile_pool(name="psd", bufs=2, space="PSUM") as psd,
            tc.tile_pool(name="psy", bufs=4, space="PSUM") as psy,
        ):
            # ---- input DMAs: program order == ring FIFO order ----
            # sync+scalar rings carry ONLY the stream, piece-interleaved:
            # b0 gram, xc0, b1 gram, xc1.  Each 512KB chunk is split into
            # two 256KB pieces on opposite rings so both rings work on the
            # same chunk and the PE's per-piece waits stay ~0.35us.
            xg_tiles = [[], []]
            qs = [nc.sync, nc.scalar]
            # ALL input descriptors ride the sync ring alone (one HWDGE ring
            # stripes across all DMA engines, and 256KB/issue keeps issue
            # capacity ~427GB/s above the ~360GB/s stream).  This keeps the
            # scalar queue descriptor-free: its ACT exps would otherwise sit
            # behind ring-backpressured issues until the whole stream had
            # been enqueued (~34us), stalling the batch-0 attention chain.
            inq = [nc.sync, nc.sync]

            def make_xg(b, c, pieces=PIECES):
                t = data.tile([P, CHW], FP8, tag=f"xg{b}_{c}")
                off = (b * SUB + c * CHUNK_SUB) * P
                pw = CHW // pieces
                for p in range(pieces):
                    inq[p % 2].dma_start(
                        t[:, p * pw : (p + 1) * pw],
                        xg_ext[:, off + p * pw : off + (p + 1) * pw],
                    )
                xg_tiles[b].append(t)

            xc = data.tile([P, 2 * NSH], BF, tag="xc")
            wpack = const.tile([P, WCOLS], BF, tag="wpack")
            wq = wpack[:, 0:512]
            wk = wpack[:, 512:1024]
            wv = wpack[:, 1024:1536]
            wo = wpack[:, 1536:1792]

            # wpack rides the gpsimd SWDGE ring: needed mid-stream, and it
            # must not displace gram bytes at the head of the hw rings.
            nc.gpsimd.dma_start(wpack[:], w_ext[:])

            # ALL gram first (both batches back to back: no PE famine at the
            # b0->b1 boundary), then xc0, then xc1.  Phase E runs entirely on
            # the tail, where xc arrives exactly when needed and nothing
            # mid-stream ever waits on the descriptor-clogged hw queues.
            for b in range(2):
                for c in range(NCHUNK):
                    # the DMA engines keep ~16 descriptors in flight, so the
                    # last bytes of a phase complete together at its end;
                    # finer final pieces let the tail's gram sems land sooner
                    make_xg(b, c, 4 if (b, c) == (1, NCHUNK - 1) else PIECES)
            for h in range(8):
                nc.sync.dma_start(
                    xc[:, h * NSH // 4 : (h + 1) * NSH // 4],
                    xc_ext[:, h * NSH // 4 : (h + 1) * NSH // 4],
                )

            # ---- constants ----
            bo = work.tile([P, 1], F32, tag="bo")
            nc.vector.tensor_copy(bo[:], wpack[:, 1792:1793])

            # ---- Gram accumulation (fp8 SwInterleave) ----
            # Each entry of g_parts[b] is a separate PSUM accumulation over a
            # chunk range; batch 1 splits [0,7) / [7,8) so the a = G Wq matmul
            # for the big part runs during the last chunk's data wait.
            g_tiles = {}

            def gram_chunks(b, c_lo, c_hi, acc_lo, acc_hi):
                key = (b, acc_lo)
                if key not in g_tiles:
                    g_tiles[key] = psg.tile(
                        [P, P], F32, tag="g", name=f"g_ps{b}_{acc_lo}"
                    )
                g_ps = g_tiles[key]
                n_mm = CHUNK_SUB // 2
                for c in range(c_lo, c_hi):
                    xr = xg_tiles[b][c][:].rearrange("p (m q) -> p m q", q=2 * P)
                    for j in range(n_mm):
                        # software-interleaved pair block: per partition the
                        # 256 bytes are [A_c127, B_c127, ..., A_c0, B_c0]
                        # (A/B = the two k-subtiles, columns reversed per the
                        # HW SwInterleave contract).  The weights AP streams
                        # the storage order; the ifmap AP picks plane i at
                        # stride 2.  G comes out with reversed columns,
                        # absorbed by reversing wk's rows host-side.
                        blk = xr[:, j, :]
                        lhsT = blk.rearrange("p (qq two) -> p qq two", two=2)
                        rhs = blk.rearrange("p (qq two) -> p two qq", two=2)
                        nc.tensor.matmul(
                            g_ps[:], lhsT, rhs,
                            start=(c == acc_lo and j == 0),
                            stop=(c == acc_hi - 1 and j == n_mm - 1),
                            perf_mode=DRSW,
                        )

            # ---- phase D: scores (PE), softmax (DVE/ACT), W_eff (PE) ----
            s_tiles = {}
            a_tiles = {}

            def d_a_part(b, acc_lo, start, stop):
                """cast one Gram part to bf16, accumulate a += G_part Wq."""
                g_ps = g_tiles[(b, acc_lo)]
                gbf = work.tile(
                    [P, P], BF, tag=f"gbf{b}_{acc_lo}", name=f"gbf{b}_{acc_lo}"
                )
                nc.vector.tensor_copy(gbf[:], g_ps[:])
                if b not in a_tiles:
                    a_tiles[b] = psd.tile([P, 512], F32, tag="d", name=f"a_ps{b}")
                nc.tensor.matmul(
                    a_tiles[b][:], gbf[:], wq, start=start, stop=stop
                )

            def d_scores(b):
                """a_sb cast (2 halves); S_h = a_h^T Wk_h (quadrant-packed)."""
                a_ps = a_tiles[b]
                a_sb = work.tile([P, 512], BF, tag=f"asb{b}", name=f"a_sb{b}")
                s_ps = psd.tile([P, 256], F32, tag="d", name=f"s_ps{b}")
                for hh in range(2):
                    nc.vector.tensor_copy(
                        a_sb[:, hh * 256 : (hh + 1) * 256],
                        a_ps[:, hh * 256 : (hh + 1) * 256],
                    )
                for h in range(HEADS):
                    pb = 64 * (h % 2)
                    cg = 64 * (h // 2)
                    nc.tensor.matmul(
                        s_ps[pb : pb + 64, cg : cg + 64],
                        a_sb[:, h * 64 : (h + 1) * 64],
                        wk[:, h * 64 : (h + 1) * 64],
                        start=True, stop=True,
                    )
                s_tiles[b] = s_ps

            bo_late = work.tile([P, 1], F32, tag="bo_late")

            def d_softmax(b):
                """Per-group: exp(s - max) with fused row-sum, then scale.

                ACT Exp takes bias = -max (per-partition AP) and emits the
                row sum via accum_out in the same instruction, so the chain
                is DVE(max) -> ACT(exp+sum) -> DVE(recip) -> DVE(scale),
                pipelined across the 4 head-groups.
                """
                s_ps = s_tiles[b]
                negmax = work.tile([P, 4], F32, tag=f"nm{b}", name=f"negmax{b}")
                exp_sb = work.tile([P, 256], F32, tag=f"exp{b}", name=f"exp_sb{b}")
                sums = work.tile([P, 4], F32, tag=f"sums{b}", name=f"sums{b}")
                recip = work.tile([P, 4], F32, tag=f"recip{b}", name=f"recip{b}")
                attn = work.tile([P, 256], BF, tag=f"attn{b}", name=f"attn{b}")
                nc.vector.reduce_max(
                    negmax[:],
                    s_ps[:].rearrange("p (g j) -> p g j", j=64),
                    axis=mybir.AxisListType.X,
                    negate=True,
                )
                for g in range(4):
                    cg = 64 * g
                    nc.scalar.activation(
                        exp_sb[:, cg : cg + 64],
                        s_ps[:, cg : cg + 64],
                        EXP,
                        bias=negmax[:, g : g + 1],
                        scale=1.0,
                        accum_out=sums[:, g : g + 1],
                    )
                    nc.vector.reciprocal(recip[:, g : g + 1], sums[:, g : g + 1])
                    nc.vector.tensor_scalar_mul(
                        attn[:, cg : cg + 64],
                        exp_sb[:, cg : cg + 64],
                        recip[:, g : g + 1],
                    )
                if b == 1:
                    # bo_late = bo, plus a read-dep on the last exp1 group's
                    # reciprocal: batch-0's bias-adds use it so the scheduler
                    # cannot wedge them between the batch-1 exp groups
                    nc.vector.tensor_scalar(
                        bo_late[:], bo[:], recip[:, 3:4], None,
                        op0=mybir.AluOpType.bypass,
                    )
                return attn

            def d_weff(b, attn):
                """MT_h = attn_h^T WoT_h; W_eff = wv MT.

                Monolithic: 8 quadrant matmuls, ONE [128,256] cast, 4
                accumulating matmuls, one weff cast -- two cross-engine
                hops total.  (A per-group cast<->matmul ping-pong costs 8
                hops and serializes ~4us on the tail.)
                """
                mt_ps = psd.tile([P, 256], F32, tag="d", name=f"mt_ps{b}")
                mt_sb = work.tile([P, 256], BF, tag=f"mt{b}", name=f"mt_sb{b}")
                w_ps = psd.tile([P, 64], F32, tag="d", name=f"w_ps{b}")
                weff = work.tile([P, 64], BF, tag=f"weff{b}", name=f"weff{b}")
                for h in range(HEADS):
                    pb = 64 * (h % 2)
                    cg = 64 * (h // 2)
                    nc.tensor.matmul(
                        mt_ps[pb : pb + 64, cg : cg + 64],
                        attn[pb : pb + 64, cg : cg + 64],
                        wo[pb : pb + 64, cg : cg + 64],
                        start=True, stop=True,
                    )
                nc.vector.tensor_copy(mt_sb[:], mt_ps[:])
                for g in range(4):
                    nc.tensor.matmul(
                        w_ps[:],
                        wv[:, g * P : (g + 1) * P],
                        mt_sb[:, g * 64 : (g + 1) * 64],
                        start=(g == 0), stop=(g == 3),
                    )
                nc.vector.tensor_copy(weff[:], w_ps[:])
                return weff

            def phase_e(b, weff, t_lo, t_hi):
                """y_b = W_eff_b @ x_b + b_out, two 512-col chunks per PSUM
                tile via quadrant packing (out partitions 0-63 / 64-127)."""
                for t in range(t_lo, t_hi):
                    y_ps = psy.tile([P, 512], F32, tag="y", name=f"y_ps{b}_{t}")
                    for half in (0, 1):
                        j = 2 * t + half
                        nc.tensor.matmul(
                            y_ps[64 * half : 64 * half + 64, :],
                            weff[:],
                            xc[:, b * NSH + j * 512 : b * NSH + (j + 1) * 512],
                            start=True, stop=True,
                        )
                    y_sb = ypool.tile([P, 512], BF, tag="ysb", name=f"y_sb{b}_{t}")
                    dst = out_ext[:, (b * 4 + t) * 512 : (b * 4 + t + 1) * 512]
                    if b == 0:
                        # attn1-gated bias (same value as bo): runs on the
                        # scalar queue strictly after the exp1 chain
                        nc.scalar.activation(
                            y_sb[:], y_ps[:],
                            mybir.ActivationFunctionType.Identity,
                            bias=bo_late[:, 0:1], scale=1.0,
                        )
                        # post-stream: the hw rings are free, and HWDGE
                        # issue (0.6us) beats gpsimd SWDGE gen (1.1us)
                        qs[t % 2].dma_start(dst, y_sb[:])
                    else:
                        # halves on ACT+DVE concurrently: halves the
                        # add latency on the last-pair critical path
                        nc.scalar.activation(
                            y_sb[:, 0:256], y_ps[:, 0:256],
                            mybir.ActivationFunctionType.Identity,
                            bias=bo[:, 0:1], scale=1.0,
                        )
                        nc.vector.tensor_scalar_add(
                            y_sb[:, 256:512], y_ps[:, 256:512], bo[:, 0:1]
                        )
                        qs[t % 2].dma_start(dst, y_sb[:])

            # ---- PE program order ----
            # gram0 -> gram1 back to back (stream-paced, no boundary gap:
            # D0's scores slot in after gram1's first chunk so the gbf0 cast
            # latency hides under data-paced matmuls).  The whole back half
            # (weff0/E0/weff1/E1) runs on the tail: weff0+E0 cover the
            # softmax1 ACT/DVE chain, and xc0/xc1 arrive (in that order)
            # right as phase E consumes them.
            # Batch-0 chunks consumed in order [2,0,1,3..7]: the PE's first
            # matmul waits for chunk 2's arrival (~+2.6us).  The HW duty
            # throttle demotes the clock at PE-onset + ~31-36us, so the
            # delayed onset shifts the half-clock window off the tail; the
            # queued chunks 0-1 drain in the PE's mid-stream slack.
            gram_chunks(0, 4, 5, 4, NCHUNK)
            gram_chunks(0, 0, 4, 4, NCHUNK)
            gram_chunks(0, 5, NCHUNK, 4, NCHUNK)
            gram_chunks(1, 0, 1, 0, NCHUNK - 1)
            d_a_part(0, 4, True, True)
            d_scores(0)
            attn0 = d_softmax(0)
            gram_chunks(1, 1, NCHUNK - 1, 0, NCHUNK - 1)
            weff0 = d_weff(0, attn0)       # fills the last chunk's data wait
            d_a_part(1, 0, True, False)    # a1 += G1[chunks 0-6] Wq, ditto
            gram_chunks(1, NCHUNK - 1, NCHUNK, NCHUNK - 1, NCHUNK)
            d_a_part(1, NCHUNK - 1, False, True)
            d_scores(1)
            attn1 = d_softmax(1)
            phase_e(0, weff0, 0, 4)        # covers the softmax1 ACT/DVE chain
            weff1 = d_weff(1, attn1)
            phase_e(1, weff1, 0, 4)

    nc.compile()
    return nc


def _get_nc():
    global _CACHED_NC
    if _CACHED_NC is None:
        _CACHED_NC = build_nc()
    return _CACHED_NC


def make_in_maps(x, w_qkv, w_out, b_out):
    x = np.ascontiguousarray(x, dtype=np.float32)
    w_qkv = np.asarray(w_qkv, dtype=np.float32)
    w_out = np.asarray(w_out, dtype=np.float32)
    b_out = np.asarray(b_out, dtype=np.float32)
    xf = x.reshape(2, P, N_TOT)

    # full x, fp8, DoubleRowSwInterleave layout: subtile pairs (2t, 2t+1)
    # interleaved per column with columns reversed:
    # [p, (b, t, qq, which)] where element = x^T[subtile 2t+which][p, 127-qq]
    arr = (
        xf.transpose(0, 2, 1)            # (2, n, c)
        .reshape(2, SUB, P, P)           # (2, m, p, c)
    )
    inter = np.stack(
        [arr[:, 0::2, :, ::-1], arr[:, 1::2, :, ::-1]], axis=-1
    )                                    # (2, t, p, qq, which)
    xg_h = np.ascontiguousarray(
        inter.transpose(2, 0, 1, 3, 4).reshape(P, 2 * SUB * P)
    ).astype(f8)

    wpack = np.zeros((P, WCOLS), np.float32)
    wpack[:, 0:512] = w_qkv[:512].T * SCALE
    # rows reversed: the SwInterleave Gram produces G with reversed columns,
    # so a = G' Wq has reversed rows; reversing wk's contraction rows undoes
    # it exactly (G is symmetric).
    wpack[:, 512:1024] = w_qkv[512:1024].T[::-1, :]
    wpack[:, 1024:1536] = (
        (w_qkv[1024:] / N_TOT).reshape(4, P, P).transpose(1, 0, 2).reshape(P, 512)
    )
    for h in range(HEADS):
        wpack[
            64 * (h % 2) : 64 * (h % 2) + 64,
            1536 + 64 * (h // 2) : 1536 + 64 * (h // 2) + 64,
        ] = w_out[:, h * 64 : (h + 1) * 64].T
    wpack[:, 1792] = np.concatenate([b_out, b_out])
    wpack_h = wpack.astype(bf16)

    in_maps = []
    for c in range(NCORES):
        # own output shard, bf16, [c, (b, n)]
        xc_h = np.ascontiguousarray(
            xf[:, :, c * NSH : (c + 1) * NSH].transpose(1, 0, 2).reshape(P, 2 * NSH)
        ).astype(bf16)
        in_maps.append({"xg": xg_h, "xc": xc_h, "wpack": wpack_h})
    return in_maps


def assemble_output(results):
    # out layout: [p = 64*half + row, (b, pair t, 512)]; spatial column of
    # (b, t, half, col) is shard_base + (2t + half)*512 + col.
    y = np.empty((2, 64, N_TOT), np.float32)
    for c in range(NCORES):
        o = np.asarray(results[c]["out"]).astype(np.float32)  # [128, 4096]
        for b in range(2):
            for t in range(4):
                blk = o[:, (b * 4 + t) * 512 : (b * 4 + t + 1) * 512]
                y[b, :, c * NSH + 2 * t * 512 : c * NSH + (2 * t + 1) * 512] = blk[:64]
                y[b, :, c * NSH + (2 * t + 1) * 512 : c * NSH + (2 * t + 2) * 512] = (
                    blk[64:]
                )
    return y.reshape(2, 64, 32, 32, 32)


def kernel(**inputs):
    in_maps = make_in_maps(
        inputs["x"], inputs["w_qkv"], inputs["w_out"], inputs["b_out"]
    )
    nc = _get_nc()
    res = run_bass_kernel_spmd(nc, in_maps, core_ids=list(range(NCORES)))
    return assemble_output(res.results)


# revision 24
# speedup vs baseline: 1.0494x; 1.0494x over previous
"""Trainium2 Bass kernel for nn_Attention (channel-attention, 8 NeuronCores).

Algorithm (algebraically identical to the reference):
  The attention contracts over the spatial axis n = 32*32*32 = 32768, and the
  attention matrices are tiny (64x64 per head).  Everything collapses around
  the per-batch Gram matrix G_b = x_b @ x_b^T (128x128):

    scores_bh = scale * Wq_h G_b Wk_h^T            (tiny)
    attn      = softmax(scores)                     (tiny)
    W_eff_b   = (1/n) * sum_h Wout_h attn_bh Wv_h   (64x128, tiny)
    y_b       = W_eff_b @ x_b + b_out               (the only other big matmul)

  Sharding: NO collectives (an ncfw collective costs 60-80us of firmware
  wakeup on this stack, dwarfing the kernel).  Every core receives the FULL
  x in fp8-e4m3 [n, c] layout (8 MB) and computes the complete Gram
  redundantly (fp8 is harmless: the Gram contracts over 32768 samples), plus
  its own 1/8 spatial shard in bf16 [c, n] layout (2 MB) for the y matmul.

  Performance model (from NTFF/perfetto analysis, iterated on HW):
  - The input stream is the wall: ~10.9 MB at ~360 GB/s = ~30 us/core.
  - The PE consumes a fp8 SwInterleave Gram pair (256 spatial rows) every
    ~80 ns (the production LDWEIGHTS+MATMUL issue floor at N=128), i.e.
    ~20.7 us of Gram work that must ride inside the stream.
  - The HW power manager grants the PE only ~20 us of full clock per run
    (HAM k=8/8 windows in the NTFF), then duty-throttles to half; every
    wasted PE cycle stretches the run at 2x.  So: no filler matmuls, and
    total PE work is kept minimal.
  Structure:
  - 256KB-piece streaming of ALL gram bytes first (b0 then b1 back to
    back), then the xc shards, so the tail-needed data arrives last and
    the PE is never starved at a batch boundary.  All input descriptors
    ride the sync ring ALONE: one HWDGE ring sustains the full stream,
    and the scalar queue stays descriptor-free -- otherwise its ACT exps
    sit behind ring-backpressured issues until the whole stream has been
    enqueued, stalling the batch-0 attention chain (measured +5 us).
  - Batch-1's Gram is split [chunks 0-6 | chunk 7] into two PSUM tiles so
    a1 = G Wq for the big part runs during the last chunk's data wait.
  - The softmax is pipelined per head-group: ACT exp carries bias=-max
    and accum_out=row-sum in ONE instruction, so the chain per group is
    DVE(max) -> ACT(exp+sum) -> DVE(recip, scale) -> PE(mt).
  - W_eff is computed monolithically (8 quadrant matmuls, one [128,256]
    cast, 4 accumulating matmuls): two cross-engine hops total.
  - Phase E packs two 512-col output chunks into one [128, 512] PSUM tile
    (PE quadrant packing), halving the bias-add and output-DMA count; the
    bias-adds alternate ACT/DVE so consecutive pairs parallelize, and
    batch-0's adds are dependency-gated behind the batch-1 exp chain
    (bo_late bypass) so the scheduler cannot wedge them into it.
  - Both phase-E's run on the tail, where weff0+E0 cover the softmax1
    ACT/DVE chain and the xc shards arrive exactly as consumed.
"""

import numpy as np
import ml_dtypes

import concourse.bass as bass
import concourse.bacc as bacc
import concourse.mybir as mybir
import concourse.tile as tile
from concourse.bass_utils import run_bass_kernel_spmd

NCORES = 8
P = 128
N_TOT = 32 * 32 * 32          # 32768 spatial points
NSH = N_TOT // NCORES         # 4096 per core per batch (output shard)
SUB = N_TOT // P              # 256 fp8 k-subtiles per batch
CHUNK_SUB = 32                # subtiles per DMA chunk (512 KB)
NCHUNK = SUB // CHUNK_SUB     # 8 chunks per batch
CHW = CHUNK_SUB * P           # 4096 fp8 free columns per chunk
PIECES = 2                    # DMA pieces per chunk (256 KB each)
HEADS = 8
DH = 64
SCALE = DH ** -0.5
WCOLS = 512 + 512 + 512 + 256 + 1  # packed weights: wq|wk|wv|wo|bo
WARM_START = 0                # PE warm-keepers (OFF: the HW throttle is a
WARM_MID = 0                  # utilization budget -- idle EARNS credit, so
WARM_TAIL = 0                 # fillers burn it and stretch the run)
BF = mybir.dt.bfloat16
F32 = mybir.dt.float32
FP8 = mybir.dt.float8e4
DR = mybir.MatmulPerfMode.DoubleRow
DRSW = mybir.MatmulPerfMode.DoubleRowSwInterleave
EXP = mybir.ActivationFunctionType.Exp
bf16 = ml_dtypes.bfloat16
f8 = ml_dtypes.float8_e4m3

_CACHED_NC = None


class _TrimmedTileContext(tile.TileContext):
    """TileContext with a minimal exit sequence.

    The stock exit is drain -> barrier -> sem-clear -> barrier; the
    barrier + clear lower to an EVSEM butterfly measured at ~7us (every
    engine walks the 27-sem global clock).  For a single-shot kernel the
    Sync drain with global-clock waits already gates completion on every
    DMA and engine op, each engine halts in-order after its last
    scheduled instruction, and the engine preamble re-initializes the
    semaphore file on the next execution (verified: back-to-back
    executions of the same loaded NEFF stay correct).  So keep only the
    drain.
    """

    def _drain_and_barrier(self, tick_clock, wait_clock):
        from concourse.vector_clock import ScopedClock

        drain_inst = self.nc.sync.drain()
        wait_clock.add_sem_waits(
            drain_inst.ins, ScopedClock({None: tick_clock.global_clock})
        )
        popped = self.nc._tile_sem_poison_stack.pop()
        assert popped is self._sem_poison


def build_nc():
    # The stock Bass init ends with const-AP memsets guarded by a second
    # all-engine barrier; the consts are unused here and the barrier adds
    # ~2us of start-up serialization, so skip that one barrier only.
    orig_barrier = bass.Bass.all_engine_barrier
    bass.Bass.all_engine_barrier = lambda self: None
    try:
        nc = bacc.Bacc(
            "TRN2", target_bir_lowering=False, debug=False, num_devices=NCORES
        )
    finally:
        bass.Bass.all_engine_barrier = orig_barrier

    # full x, fp8, [p, (b, m, c)] DoubleRow layout: subtile m holds spatial
    # rows m*128..m*128+127 of batch b, channels on the innermost axis.
    xg_ext = nc.dram_tensor("xg", [P, 2 * SUB * P], FP8, kind="ExternalInput")
    # own output shard, bf16, [c, (b, n)] layout for the y matmul
    xc_ext = nc.dram_tensor("xc", [P, 2 * NSH], BF, kind="ExternalInput")
    w_ext = nc.dram_tensor("wpack", [P, WCOLS], BF, kind="ExternalInput")
    # y out, bf16: partition = (chunk-half, row), free = (b, pair, 512)
    out_ext = nc.dram_tensor("out", [P, NSH], BF, kind="ExternalOutput")

    with _TrimmedTileContext(nc) as tc:
        with (
            tc.tile_pool(name="const", bufs=1) as const,
            tc.tile_pool(name="data", bufs=1) as data,
            tc.tile_pool(name="work", bufs=1) as work,
            tc.tile_pool(name="ypool", bufs=8) as ypool,
            tc.tile_pool(name="psg", bufs=2, space="PSUM") as psg,
            tc.tile_pool(name="psd", bufs=2, space="PSUM") as psd,
            tc.tile_pool(name="psy", bufs=4, space="PSUM") as psy,
        ):
            # ---- input DMAs: program order == ring FIFO order ----
            # sync+scalar rings carry ONLY the stream, piece-interleaved:
            # b0 gram, xc0, b1 gram, xc1.  Each 512KB chunk is split into
            # two 256KB pieces on opposite rings so both rings work on the
            # same chunk and the PE's per-piece waits stay ~0.35us.
            xg_tiles = [[], []]
            qs = [nc.sync, nc.scalar]
            # ALL input descriptors ride the sync ring alone (one HWDGE ring
            # stripes across all DMA engines, and 256KB/issue keeps issue
            # capacity ~427GB/s above the ~360GB/s stream).  This keeps the
            # scalar queue descriptor-free: its ACT exps would otherwise sit
            # behind ring-backpressured issues until the whole stream had
            # been enqueued (~34us), stalling the batch-0 attention chain.
            inq = [nc.sync, nc.sync]

            def make_xg(b, c, pieces=PIECES):
                t = data.tile([P, CHW], FP8, tag=f"xg{b}_{c}")
                off = (b * SUB + c * CHUNK_SUB) * P
                pw = CHW // pieces
                for p in range(pieces):
                    inq[p % 2].dma_start(
                        t[:, p * pw : (p + 1) * pw],
                        xg_ext[:, off + p * pw : off + (p + 1) * pw],
                    )
                xg_tiles[b].append(t)

            xc = data.tile([P, 2 * NSH], BF, tag="xc")
            wpack = const.tile([P, WCOLS], BF, tag="wpack")
            wq = wpack[:, 0:512]
            wk = wpack[:, 512:1024]
            wv = wpack[:, 1024:1536]
            wo = wpack[:, 1536:1792]

            # wpack rides the gpsimd SWDGE ring: needed mid-stream, and it
            # must not displace gram bytes at the head of the hw rings.
            nc.gpsimd.dma_start(wpack[:], w_ext[:])

            # ALL gram first (both batches back to back: no PE famine at the
            # b0->b1 boundary), then xc0, then xc1.  Phase E runs entirely on
            # the tail, where xc arrives exactly when needed and nothing
            # mid-stream ever waits on the descriptor-clogged hw queues.
            for b in range(2):
                for c in range(NCHUNK):
                    # the DMA engines keep ~16 descriptors in flight, so the
                    # last bytes of a phase complete together at its end;
                    # finer final pieces let the tail's gram sems land sooner
                    make_xg(b, c, 4 if (b, c) == (1, NCHUNK - 1) else PIECES)
            for h in range(8):
                nc.sync.dma_start(
                    xc[:, h * NSH // 4 : (h + 1) * NSH // 4],
                    xc_ext[:, h * NSH // 4 : (h + 1) * NSH // 4],
                )

            # ---- constants ----
            bo = work.tile([P, 1], F32, tag="bo")
            nc.vector.tensor_copy(bo[:], wpack[:, 1792:1793])

            # ---- Gram accumulation (fp8 SwInterleave) ----
            # Each entry of g_parts[b] is a separate PSUM accumulation over a
            # chunk range; batch 1 splits [0,7) / [7,8) so the a = G Wq matmul
            # for the big part runs during the last chunk's data wait.
            g_tiles = {}

            def gram_chunks(b, c_lo, c_hi, acc_lo, acc_hi):
                key = (b, acc_lo)
                if key not in g_tiles:
                    g_tiles[key] = psg.tile(
                        [P, P], F32, tag="g", name=f"g_ps{b}_{acc_lo}"
                    )
                g_ps = g_tiles[key]
                n_mm = CHUNK_SUB // 2
                for c in range(c_lo, c_hi):
                    xr = xg_tiles[b][c][:].rearrange("p (m q) -> p m q", q=2 * P)
                    for j in range(n_mm):
                        # software-interleaved pair block: per partition the
                        # 256 bytes are [A_c127, B_c127, ..., A_c0, B_c0]
                        # (A/B = the two k-subtiles, columns reversed per the
                        # HW SwInterleave contract).  The weights AP streams
                        # the storage order; the ifmap AP picks plane i at
                        # stride 2.  G comes out with reversed columns,
                        # absorbed by reversing wk's rows host-side.
                        blk = xr[:, j, :]
                        lhsT = blk.rearrange("p (qq two) -> p qq two", two=2)
                        rhs = blk.rearrange("p (qq two) -> p two qq", two=2)
                        nc.tensor.matmul(
                            g_ps[:], lhsT, rhs,
                            start=(c == acc_lo and j == 0),
                            stop=(c == acc_hi - 1 and j == n_mm - 1),
                            perf_mode=DRSW,
                        )

            # ---- phase D: scores (PE), softmax (DVE/ACT), W_eff (PE) ----
            s_tiles = {}
            a_tiles = {}

            def d_a_part(b, acc_lo, start, stop):
                """cast one Gram part to bf16, accumulate a += G_part Wq."""
                g_ps = g_tiles[(b, acc_lo)]
                gbf = work.tile(
                    [P, P], BF, tag=f"gbf{b}_{acc_lo}", name=f"gbf{b}_{acc_lo}"
                )
                nc.vector.tensor_copy(gbf[:], g_ps[:])
                if b not in a_tiles:
                    a_tiles[b] = psd.tile([P, 512], F32, tag="d", name=f"a_ps{b}")
                nc.tensor.matmul(
                    a_tiles[b][:], gbf[:], wq, start=start, stop=stop
                )

            def d_scores(b):
                """a_sb cast (2 halves); S_h = a_h^T Wk_h (quadrant-packed)."""
                a_ps = a_tiles[b]
                a_sb = work.tile([P, 512], BF, tag=f"asb{b}", name=f"a_sb{b}")
                s_ps = psd.tile([P, 256], F32, tag="d", name=f"s_ps{b}")
                for hh in range(2):
                    nc.vector.tensor_copy(
                        a_sb[:, hh * 256 : (hh + 1) * 256],
                        a_ps[:, hh * 256 : (hh + 1) * 256],
                    )
                for h in range(HEADS):
                    pb = 64 * (h % 2)
                    cg = 64 * (h // 2)
                    nc.tensor.matmul(
                        s_ps[pb : pb + 64, cg : cg + 64],
                        a_sb[:, h * 64 : (h + 1) * 64],
                        wk[:, h * 64 : (h + 1) * 64],
                        start=True, stop=True,
                    )
                s_tiles[b] = s_ps

            bo_late = work.tile([P, 1], F32, tag="bo_late")

            def d_softmax(b):
                """Per-group: exp(s - max) with fused row-sum, then scale.

                ACT Exp takes bias = -max (per-partition AP) and emits the
                row sum via accum_out in the same instruction, so the chain
                is DVE(max) -> ACT(exp+sum) -> DVE(recip) -> DVE(scale),
                pipelined across the 4 head-groups.
                """
                s_ps = s_tiles[b]
                negmax = work.tile([P, 4], F32, tag=f"nm{b}", name=f"negmax{b}")
                exp_sb = work.tile([P, 256], F32, tag=f"exp{b}", name=f"exp_sb{b}")
                sums = work.tile([P, 4], F32, tag=f"sums{b}", name=f"sums{b}")
                recip = work.tile([P, 4], F32, tag=f"recip{b}", name=f"recip{b}")
                attn = work.tile([P, 256], BF, tag=f"attn{b}", name=f"attn{b}")
                nc.vector.reduce_max(
                    negmax[:],
                    s_ps[:].rearrange("p (g j) -> p g j", j=64),
                    axis=mybir.AxisListType.X,
                    negate=True,
                )
                for g in range(4):
                    cg = 64 * g
                    nc.scalar.activation(
                        exp_sb[:, cg : cg + 64],
                        s_ps[:, cg : cg + 64],
                        EXP,
                        bias=negmax[:, g : g + 1],
                        scale=1.0,
                        accum_out=sums[:, g : g + 1],
                    )
                    nc.vector.reciprocal(recip[:, g : g + 1], sums[:, g : g + 1])
                    nc.vector.tensor_scalar_mul(
                        attn[:, cg : cg + 64],
                        exp_sb[:, cg : cg + 64],
                        recip[:, g : g + 1],
                    )
                if b == 1:
                    # bo_late = bo, plus a read-dep on the last exp1 group's
                    # reciprocal: batch-0's bias-adds use it so the scheduler
                    # cannot wedge them between the batch-1 exp groups
                    nc.vector.tensor_scalar(
                        bo_late[:], bo[:], recip[:, 3:4], None,
                        op0=mybir.AluOpType.bypass,
                    )
                return attn

            def d_weff(b, attn):
                """MT_h = attn_h^T WoT_h; W_eff = wv MT.

                Monolithic: 8 quadrant matmuls, ONE [128,256] cast, 4
                accumulating matmuls, one weff cast -- two cross-engine
                hops total.  (A per-group cast<->matmul ping-pong costs 8
                hops and serializes ~4us on the tail.)
                """
                mt_ps = psd.tile([P, 256], F32, tag="d", name=f"mt_ps{b}")
                mt_sb = work.tile([P, 256], BF, tag=f"mt{b}", name=f"mt_sb{b}")
                w_ps = psd.tile([P, 64], F32, tag="d", name=f"w_ps{b}")
                weff = work.tile([P, 64], BF, tag=f"weff{b}", name=f"weff{b}")
                for h in range(HEADS):
                    pb = 64 * (h % 2)
                    cg = 64 * (h // 2)
                    nc.tensor.matmul(
                        mt_ps[pb : pb + 64, cg : cg + 64],
                        attn[pb : pb + 64, cg : cg + 64],
                        wo[pb : pb + 64, cg : cg + 64],
                        start=True, stop=True,
                    )
                for hh in range(2):
                    nc.vector.tensor_copy(
                        mt_sb[:, hh * 128 : (hh + 1) * 128],
                        mt_ps[:, hh * 128 : (hh + 1) * 128],
                    )
                for g in range(4):
                    nc.tensor.matmul(
                        w_ps[:],
                        wv[:, g * P : (g + 1) * P],
                        mt_sb[:, g * 64 : (g + 1) * 64],
                        start=(g == 0), stop=(g == 3),
                    )
                nc.vector.tensor_copy(weff[:], w_ps[:])
                return weff

            def phase_e(b, weff, t_lo, t_hi):
                """y_b = W_eff_b @ x_b + b_out, two 512-col chunks per PSUM
                tile via quadrant packing (out partitions 0-63 / 64-127)."""
                for t in range(t_lo, t_hi):
                    y_ps = psy.tile([P, 512], F32, tag="y", name=f"y_ps{b}_{t}")
                    for half in (0, 1):
                        j = 2 * t + half
                        nc.tensor.matmul(
                            y_ps[64 * half : 64 * half + 64, :],
                            weff[:],
                            xc[:, b * NSH + j * 512 : b * NSH + (j + 1) * 512],
                            start=True, stop=True,
                        )
                    y_sb = ypool.tile([P, 512], BF, tag="ysb", name=f"y_sb{b}_{t}")
                    dst = out_ext[:, (b * 4 + t) * 512 : (b * 4 + t + 1) * 512]
                    if b == 0:
                        # attn1-gated bias (same value as bo): runs on the
                        # scalar queue strictly after the exp1 chain
                        nc.scalar.activation(
                            y_sb[:], y_ps[:],
                            mybir.ActivationFunctionType.Identity,
                            bias=bo_late[:, 0:1], scale=1.0,
                        )
                        # post-stream: the hw rings are free, and HWDGE
                        # issue (0.6us) beats gpsimd SWDGE gen (1.1us)
                        qs[t % 2].dma_start(dst, y_sb[:])
                    else:
                        # halves on ACT+DVE concurrently: halves the
                        # add latency on the last-pair critical path
                        nc.scalar.activation(
                            y_sb[:, 0:256], y_ps[:, 0:256],
                            mybir.ActivationFunctionType.Identity,
                            bias=bo[:, 0:1], scale=1.0,
                        )
                        nc.vector.tensor_scalar_add(
                            y_sb[:, 256:512], y_ps[:, 256:512], bo[:, 0:1]
                        )
                        qs[t % 2].dma_start(dst, y_sb[:])

            # ---- PE program order ----
            # gram0 -> gram1 back to back (stream-paced, no boundary gap:
            # D0's scores slot in after gram1's first chunk so the gbf0 cast
            # latency hides under data-paced matmuls).  The whole back half
            # (weff0/E0/weff1/E1) runs on the tail: weff0+E0 cover the
            # softmax1 ACT/DVE chain, and xc0/xc1 arrive (in that order)
            # right as phase E consumes them.
            # Batch-0 chunks consumed in order [2,0,1,3..7]: the PE's first
            # matmul waits for chunk 2's arrival (~+2.6us).  The HW duty
            # throttle demotes the clock at PE-onset + ~31-36us, so the
            # delayed onset shifts the half-clock window off the tail; the
            # queued chunks 0-1 drain in the PE's mid-stream slack.
            gram_chunks(0, 2, 3, 2, NCHUNK)
            gram_chunks(0, 0, 2, 2, NCHUNK)
            gram_chunks(0, 3, NCHUNK, 2, NCHUNK)
            gram_chunks(1, 0, 1, 0, NCHUNK - 1)
            d_a_part(0, 2, True, True)
            d_scores(0)
            attn0 = d_softmax(0)
            gram_chunks(1, 1, NCHUNK - 1, 0, NCHUNK - 1)
            weff0 = d_weff(0, attn0)       # fills the last chunk's data wait
            d_a_part(1, 0, True, False)    # a1 += G1[chunks 0-6] Wq, ditto
            gram_chunks(1, NCHUNK - 1, NCHUNK, NCHUNK - 1, NCHUNK)
            d_a_part(1, NCHUNK - 1, False, True)
            d_scores(1)
            attn1 = d_softmax(1)
            phase_e(0, weff0, 0, 4)        # covers the softmax1 ACT/DVE chain
            weff1 = d_weff(1, attn1)
            phase_e(1, weff1, 0, 4)

    nc.compile()
    return nc


def _get_nc():
    global _CACHED_NC
    if _CACHED_NC is None:
        _CACHED_NC = build_nc()
    return _CACHED_NC


def make_in_maps(x, w_qkv, w_out, b_out):
    x = np.ascontiguousarray(x, dtype=np.float32)
    w_qkv = np.asarray(w_qkv, dtype=np.float32)
    w_out = np.asarray(w_out, dtype=np.float32)
    b_out = np.asarray(b_out, dtype=np.float32)
    xf = x.reshape(2, P, N_TOT)

    # full x, fp8, DoubleRowSwInterleave layout: subtile pairs (2t, 2t+1)
    # interleaved per column with columns reversed:
    # [p, (b, t, qq, which)] where element = x^T[subtile 2t+which][p, 127-qq]
    arr = (
        xf.transpose(0, 2, 1)            # (2, n, c)
        .reshape(2, SUB, P, P)           # (2, m, p, c)
    )
    inter = np.stack(
        [arr[:, 0::2, :, ::-1], arr[:, 1::2, :, ::-1]], axis=-1
    )                                    # (2, t, p, qq, which)
    xg_h = np.ascontiguousarray(
        inter.transpose(2, 0, 1, 3, 4).reshape(P, 2 * SUB * P)
    ).astype(f8)

    wpack = np.zeros((P, WCOLS), np.float32)
    wpack[:, 0:512] = w_qkv[:512].T * SCALE
    # rows reversed: the SwInterleave Gram produces G with reversed columns,
    # so a = G' Wq has reversed rows; reversing wk's contraction rows undoes
    # it exactly (G is symmetric).
    wpack[:, 512:1024] = w_qkv[512:1024].T[::-1, :]
    wpack[:, 1024:1536] = (
        (w_qkv[1024:] / N_TOT).reshape(4, P, P).transpose(1, 0, 2).reshape(P, 512)
    )
    for h in range(HEADS):
        wpack[
            64 * (h % 2) : 64 * (h % 2) + 64,
            1536 + 64 * (h // 2) : 1536 + 64 * (h // 2) + 64,
        ] = w_out[:, h * 64 : (h + 1) * 64].T
    wpack[:, 1792] = np.concatenate([b_out, b_out])
    wpack_h = wpack.astype(bf16)

    in_maps = []
    for c in range(NCORES):
        # own output shard, bf16, [c, (b, n)]
        xc_h = np.ascontiguousarray(
            xf[:, :, c * NSH : (c + 1) * NSH].transpose(1, 0, 2).reshape(P, 2 * NSH)
        ).astype(bf16)
        in_maps.append({"xg": xg_h, "xc": xc_h, "wpack": wpack_h})
    return in_maps


def assemble_output(results):
    # out layout: [p = 64*half + row, (b, pair t, 512)]; spatial column of
    # (b, t, half, col) is shard_base + (2t + half)*512 + col.
    y = np.empty((2, 64, N_TOT), np.float32)
    for c in range(NCORES):
        o = np.asarray(results[c]["out"]).astype(np.float32)  # [128, 4096]
        for b in range(2):
            for t in range(4):
                blk = o[:, (b * 4 + t) * 512 : (b * 4 + t + 1) * 512]
                y[b, :, c * NSH + 2 * t * 512 : c * NSH + (2 * t + 1) * 512] = blk[:64]
                y[b, :, c * NSH + (2 * t + 1) * 512 : c * NSH + (2 * t + 2) * 512] = (
                    blk[64:]
                )
    return y.reshape(2, 64, 32, 32, 32)


def kernel(**inputs):
    in_maps = make_in_maps(
        inputs["x"], inputs["w_qkv"], inputs["w_out"], inputs["b_out"]
    )
    nc = _get_nc()
    res = run_bass_kernel_spmd(nc, in_maps, core_ids=list(range(NCORES)))
    return assemble_output(res.results)
